# revision 15
# baseline (speedup 1.0000x reference)
"""Causal self-attention (B=2, S=2048, D=1024, H=16) on 8 TRN2 NeuronCores.

Sharding: tensor-parallel over heads. Core c owns heads {2c, 2c+1} for BOTH
batches: it computes Q/K/V projections for its 2 heads (1/8 of the QKV work,
no redundancy), causal attention for its heads over all tokens (skipping
fully-masked 128x256 blocks; diagonal blocks handled by a 0/1 mask multiply
after exp), and softmax-normalizes via a ones-column appended to V (row 64 of
the PV accumulator = denominators).

The normalized per-head outputs y^T are then redistributed with a single
all-to-all over the 8 cores (bf16, 1MB total per core): core c sends
y^T[my 128 dims, tokens of core j] to each j, and receives its own 512
tokens' y for all 1024 dims. Each core then runs the output projection for
its 512 tokens locally and writes the final [512, 1024] f32 block.

All matmuls run in bf16 (full PE rate at any moving size); PSUM accumulates
f32. rel-err budget ~0.5% << 2% tolerance.
"""

import numpy as np

import concourse.bass as bass
import concourse.mybir as mybir
import concourse.tile as tile
from concourse import bacc
from concourse.bass_utils import run_bass_kernel_spmd

F32 = mybir.dt.float32
BF16 = mybir.dt.bfloat16
AF = mybir.ActivationFunctionType
ALU = mybir.AluOpType

B, S, D, H, HD = 2, 2048, 1024, 16, 64
QL = 512           # tokens output per core
NKC = D // 128     # 8 contraction chunks for the projections
QG = 256           # attention q-group width
NQG = S // QG      # 8 q-groups
TCH = 512          # token chunk width for streaming x^T
NTC = S // TCH     # 4
SCALE = 1.0 / np.sqrt(HD)

_CACHED = {}

NPBF16 = mybir.dt.np(BF16)


def build_nc():
    nc = bacc.Bacc("TRN2", target_bir_lowering=False, debug=False)

    xt0 = nc.dram_tensor("xt0", [D, S], BF16, kind="ExternalInput").ap()
    xt1 = nc.dram_tensor("xt1", [D, S], BF16, kind="ExternalInput").ap()
    wq = nc.dram_tensor("wq", [D, 128], BF16, kind="ExternalInput").ap()
    wk = nc.dram_tensor("wk", [D, 128], BF16, kind="ExternalInput").ap()
    wv = nc.dram_tensor("wv", [D, 128], BF16, kind="ExternalInput").ap()
    bq = nc.dram_tensor("bq", [128, 1], F32, kind="ExternalInput").ap()
    bk = nc.dram_tensor("bk", [128, 1], F32, kind="ExternalInput").ap()
    bv = nc.dram_tensor("bv", [1, 128], BF16, kind="ExternalInput").ap()
    wp = nc.dram_tensor("wp", [D, D], BF16, kind="ExternalInput").ap()
    bp = nc.dram_tensor("bp", [1, D], BF16, kind="ExternalInput").ap()
    maskd = nc.dram_tensor("maskd", [128, 2 * QG], BF16, kind="ExternalInput").ap()
    ones1 = nc.dram_tensor("ones1", [1, 128], BF16, kind="ExternalInput").ap()
    out = nc.dram_tensor("out", [QL, D], F32, kind="ExternalOutput").ap()

    with tile.TileContext(nc) as tc:
        _body(nc, tc, [xt0, xt1], wq, wk, wv, bq, bk, bv, wp, bp, maskd,
              ones1, out)
    nc.compile()
    return nc


def _body(nc, tc, xt, wq, wk, wv, bq, bk, bv, wp, bp, maskd, ones1, out):
    with (
        tc.tile_pool(name="const", bufs=1) as const_p,
        tc.tile_pool(name="w", bufs=1) as w_p,
        tc.tile_pool(name="qkv", bufs=1) as qkv_p,
        tc.tile_pool(name="psum", bufs=4, space="PSUM") as psum,
        tc.tile_pool(name="opsum", bufs=2, space="PSUM") as opsum,
        tc.tile_pool(name="rpsum", bufs=2, space="PSUM") as rpsum,
        tc.tile_pool(name="dram", bufs=1, space="DRAM") as dram,
    ):
        # ---------------- constants ----------------
        ones_s = const_p.tile([1, 128], BF16)
        nc.sync.dma_start(ones_s[:], ones1[:])
        mask_s = const_p.tile([128, 2 * QG], BF16)
        nc.sync.dma_start(mask_s[:], maskd[:])
        bq_s = const_p.tile([128, 1], F32)
        nc.sync.dma_start(bq_s[:], bq[:])
        bk_s = const_p.tile([128, 1], F32)
        nc.sync.dma_start(bk_s[:], bk[:])
        bv_s = const_p.tile([1, 128], BF16)
        nc.sync.dma_start(bv_s[:], bv[:])
        bp_s = const_p.tile([1, D], BF16)
        nc.sync.dma_start(bp_s[:], bp[:])

        # broadcast v-bias to all 128 token partitions via K=1 matmul
        bv_bc = const_p.tile([128, 128], BF16)
        ps_bv = psum.tile([128, 512], F32, tag="ps", name="bvbc")
        nc.tensor.matmul(ps_bv[:, 0:128], ones_s[:], bv_s[:],
                         start=True, stop=True)
        nc.vector.tensor_copy(bv_bc[:], ps_bv[:, 0:128])

        # ---------------- weights ----------------
        wq_s = w_p.tile([128, D], BF16)   # [d-chunk part, kc*128 + qcol]
        wk_s = w_p.tile([128, D], BF16)
        wv_s = w_p.tile([128, D], BF16)
        for kc in range(NKC):
            nc.sync.dma_start(wq_s[:, kc * 128:(kc + 1) * 128],
                              wq[kc * 128:(kc + 1) * 128, :])
            nc.sync.dma_start(wk_s[:, kc * 128:(kc + 1) * 128],
                              wk[kc * 128:(kc + 1) * 128, :])
            nc.sync.dma_start(wv_s[:, kc * 128:(kc + 1) * 128],
                              wv[kc * 128:(kc + 1) * 128, :])

        # ---------------- QKV projection (both batches) ----------------
        # per-token-chunk tiles so attention can start before QKV finishes
        qt = [[qkv_p.tile([128, TCH], BF16, tag=f"qt{b}_{t}",
                          name=f"qt{b}_{t}") for t in range(NTC)]
              for b in range(B)]
        kt = [[qkv_p.tile([128, TCH], BF16, tag=f"kt{b}_{t}",
                          name=f"kt{b}_{t}") for t in range(NTC)]
              for b in range(B)]
        # V natural, [token part, 4 chunks x (2 heads x 65)] per tcn
        v2 = [[qkv_p.tile([128, 4 * 130], BF16, tag=f"v{b}_{t}",
                          name=f"v{b}_{t}") for t in range(NTC)]
              for b in range(B)]
        for b in range(B):
            for t in range(NTC):
                v4 = v2[b][t][:].rearrange("p (t h e) -> p t h e", h=2, e=65)
                nc.vector.memset(v4[:, :, :, 64:65], 1.0)

        with tc.tile_pool(name="xtc", bufs=2) as xtc_p:
            for b in range(B):
                for tcn in range(NTC):
                    xc = [
                        xtc_p.tile([128, TCH], BF16, tag=f"x{kc}",
                                   name=f"x{b}_{tcn}_{kc}")
                        for kc in range(NKC)
                    ]
                    for kc in range(NKC):
                        nc.sync.dma_start(
                            xc[kc][:],
                            xt[b][kc * 128:(kc + 1) * 128,
                                  tcn * TCH:(tcn + 1) * TCH],
                        )
                    # Q^T chunk
                    ps = psum.tile([128, TCH], F32, tag="ps",
                                   name=f"q{b}_{tcn}")
                    for kc in range(NKC):
                        nc.tensor.matmul(
                            ps[:], wq_s[:, kc * 128:(kc + 1) * 128], xc[kc][:],
                            start=(kc == 0), stop=(kc == NKC - 1),
                        )
                    nc.vector.tensor_scalar(
                        qt[b][tcn][:], ps[:], bq_s[:], None, ALU.add,
                    )
                    # K^T chunk
                    ps = psum.tile([128, TCH], F32, tag="ps",
                                   name=f"k{b}_{tcn}")
                    for kc in range(NKC):
                        nc.tensor.matmul(
                            ps[:], wk_s[:, kc * 128:(kc + 1) * 128], xc[kc][:],
                            start=(kc == 0), stop=(kc == NKC - 1),
                        )
                    nc.vector.tensor_scalar(
                        kt[b][tcn][:], ps[:], bk_s[:], None, ALU.add,
                    )
                    # V natural per 128-token sub-chunk
                    for vt in range(TCH // 128):
                        vtg = tcn * (TCH // 128) + vt
                        ps = psum.tile([128, 512], F32, tag="ps",
                                       name=f"v{b}_{vtg}")
                        for kc in range(NKC):
                            nc.tensor.matmul(
                                ps[:, 0:128],
                                xc[kc][:, vt * 128:(vt + 1) * 128],
                                wv_s[:, kc * 128:(kc + 1) * 128],
                                start=(kc == 0), stop=(kc == NKC - 1),
                            )
                        v4 = v2[b][tcn][:].rearrange("p (t h e) -> p t h e",
                                                     h=2, e=65)
                        nc.vector.tensor_tensor(
                            v4[:, vt, :, 0:64],
                            ps[:, 0:128].rearrange("p (h e) -> p h e", e=64),
                            bv_bc[:].rearrange("p (h e) -> p h e", e=64),
                            ALU.add,
                        )

        # ---------------- attention ----------------
        # load wp during attention (DMA idle here)
        wp_s = w_p.tile([128, NKC * D], BF16, tag="wp", name="wp")
        for kc in range(NKC):
            nc.sync.dma_start(wp_s[:, kc * D:(kc + 1) * D],
                              wp[kc * 128:(kc + 1) * 128, :])

        ytn = [qkv_p.tile([128, S], BF16, tag=f"ytn{b}", name=f"ytn{b}")
               for b in range(B)]

        with (
            tc.tile_pool(name="pt", bufs=3) as p_p,
            tc.tile_pool(name="ys", bufs=2) as ys_p,
            tc.tile_pool(name="rec", bufs=2) as rec_p,
        ):
            for b in range(B):
                for hh in range(2):
                    e0 = hh * 64
                    ys = ys_p.tile([65, S], F32, tag="ys", name=f"ys{b}_{hh}")
                    for gp in range(NQG // 2):   # o-tile covers 2 q-groups
                        op = opsum.tile([65, 2 * QG], F32, tag="o",
                                        name=f"o{b}_{hh}_{gp}")
                        for gh in range(2):
                            g = 2 * gp + gh
                            q0 = g * QG
                            nkv = 2 * g + 2   # kv chunks for this q-group
                            for pr in range(nkv // 2):  # 2 kv-chunks per s-tile
                                sp = psum.tile([128, 2 * QG], F32, tag="ps",
                                               name=f"s{b}_{hh}_{g}_{pr}")
                                pt = p_p.tile([128, 2 * QG], BF16, tag="pt",
                                              name=f"p{b}_{hh}_{g}_{pr}")
                                for kk in range(2):
                                    kvc = 2 * pr + kk
                                    nc.tensor.matmul(
                                        sp[:, kk * QG:(kk + 1) * QG],
                                        kt[b][kvc // 4][e0:e0 + 64,
                                                        (kvc % 4) * 128:
                                                        (kvc % 4 + 1) * 128],
                                        qt[b][g // 2][e0:e0 + 64,
                                                      (q0 % TCH):
                                                      (q0 % TCH) + QG],
                                        start=True, stop=True,
                                    )
                                nc.scalar.activation(pt[:], sp[:], AF.Exp)
                                if pr == nkv // 2 - 1:
                                    # diagonal pair: zero the upper triangle
                                    nc.vector.tensor_tensor(
                                        pt[:], pt[:], mask_s[:], ALU.mult,
                                    )
                                for kk in range(2):
                                    kvc = 2 * pr + kk
                                    v4 = v2[b][kvc // 4][:].rearrange(
                                        "p (t h e) -> p t h e", h=2, e=65)
                                    nc.tensor.matmul(
                                        op[:, gh * QG:(gh + 1) * QG],
                                        v4[:, kvc % 4, hh, :],
                                        pt[:, kk * QG:(kk + 1) * QG],
                                        start=(pr == 0 and kk == 0),
                                        stop=(pr == nkv // 2 - 1 and kk == 1),
                                    )
                        nc.vector.tensor_copy(
                            ys[:, gp * 2 * QG:(gp + 1) * 2 * QG], op[:])
                    # normalize: rec = 1/den, broadcast via K=1 matmul, mult
                    rec = rec_p.tile([1, S], BF16, tag="rec",
                                     name=f"rec{b}_{hh}")
                    with nc.allow_low_precision(reason="bf16 softmax denom"):
                        nc.vector.reciprocal(rec[:], ys[64:65, :])
                    for qq in range(4):
                        rp = rpsum.tile([64, 512], F32, tag="rp",
                                        name=f"rp{b}_{hh}_{qq}")
                        nc.tensor.matmul(
                            rp[:], ones_s[:, 0:64],
                            rec[:, qq * 512:(qq + 1) * 512],
                            start=True, stop=True,
                        )
                        nc.vector.tensor_tensor(
                            ytn[b][e0:e0 + 64, qq * 512:(qq + 1) * 512],
                            ys[0:64, qq * 512:(qq + 1) * 512],
                            rp[:], ALU.mult,
                        )

        # ---------------- all-to-all redistribution of y ----------------
        a2a_in = dram.tile([8, 128, QL], BF16)
        a2a_out = dram.tile([8, 128, QL], BF16)
        for j in range(8):
            nc.sync.dma_start(
                a2a_in[j],
                ytn[j // 4][:, (j % 4) * QL:(j % 4 + 1) * QL],
            )
        nc.gpsimd.collective_compute(
            "AllToAll",
            ALU.bypass,
            replica_groups=[[0, 1, 2, 3, 4, 5, 6, 7]],
            ins=[a2a_in[:]],
            outs=[a2a_out[:]],
        )
        yg = qkv_p.tile([128, 8 * QL], BF16, tag="yg", name="yg")
        for r in range(8):
            nc.sync.dma_start(yg[:, r * QL:(r + 1) * QL], a2a_out[r])

        # ---------------- output projection ----------------
        with tc.tile_pool(name="outp", bufs=3) as out_p:
            for tc4 in range(4):
                for n2 in range(2):
                    ps = psum.tile([128, 512], F32, tag="ps",
                                   name=f"op{tc4}_{n2}")
                    for r in range(8):
                        nc.tensor.matmul(
                            ps[:],
                            yg[:, r * QL + tc4 * 128:r * QL + (tc4 + 1) * 128],
                            wp_s[:, r * D + n2 * 512:r * D + (n2 + 1) * 512],
                            start=(r == 0), stop=False,
                        )
                    nc.tensor.matmul(
                        ps[:], ones_s[:], bp_s[:, n2 * 512:(n2 + 1) * 512],
                        start=False, stop=True,
                    )
                    ot = out_p.tile([128, 512], F32, tag="ot",
                                    name=f"ot{tc4}_{n2}")
                    nc.vector.tensor_copy(ot[:], ps[:])
                    nc.sync.dma_start(
                        out[tc4 * 128:(tc4 + 1) * 128,
                            n2 * 512:(n2 + 1) * 512],
                        ot[:],
                    )


def _host_inputs(x, w_attn, b_attn, w_proj, b_proj):
    """Build the 8 per-core input maps."""
    x = np.asarray(x, np.float32)
    w_attn = np.asarray(w_attn, np.float32)
    b_attn = np.asarray(b_attn, np.float32)
    w_proj = np.asarray(w_proj, np.float32)
    b_proj = np.asarray(b_proj, np.float32)

    xt0 = np.ascontiguousarray(x[0].T).astype(NPBF16)
    xt1 = np.ascontiguousarray(x[1].T).astype(NPBF16)
    wpp = np.ascontiguousarray(w_proj).astype(NPBF16)
    bp1 = b_proj.reshape(1, D).astype(NPBF16)
    ones1 = np.ones((1, 128), NPBF16)

    # diagonal-pair 0/1 mask: cols [0:256] kv-offset 0..127, [256:512] 128..255
    p = np.arange(128)[:, None]
    q = np.arange(QG)[None, :]
    maskd = np.concatenate(
        [(p <= q).astype(np.float32), (p + 128 <= q).astype(np.float32)],
        axis=1,
    ).astype(NPBF16)

    in_maps = []
    for c in range(8):
        c0 = 128 * c
        wqc = (w_attn[:, c0:c0 + 128] * SCALE).astype(NPBF16)
        wkc = w_attn[:, D + c0:D + c0 + 128].astype(NPBF16)
        wvc = w_attn[:, 2 * D + c0:2 * D + c0 + 128].astype(NPBF16)
        bqc = (b_attn[c0:c0 + 128] * SCALE).astype(np.float32).reshape(128, 1)
        bkc = b_attn[D + c0:D + c0 + 128].astype(np.float32).reshape(128, 1)
        bvc = b_attn[2 * D + c0:2 * D + c0 + 128].reshape(1, 128).astype(NPBF16)
        in_maps.append(
            {
                "xt0": xt0, "xt1": xt1,
                "wq": wqc, "wk": wkc, "wv": wvc,
                "bq": bqc, "bk": bkc, "bv": bvc,
                "wp": wpp, "bp": bp1,
                "maskd": maskd, "ones1": ones1,
            }
        )
    return in_maps


def _assemble_full(outs):
    full = np.empty((B, S, D), np.float32)
    for c in range(8):
        b, cq = c // 4, c % 4
        full[b, cq * QL:(cq + 1) * QL] = outs[c]
    return full


def kernel(x, w_attn, b_attn, w_proj, b_proj):
    if "nc" not in _CACHED:
        _CACHED["nc"] = build_nc()
    nc = _CACHED["nc"]
    in_maps = _host_inputs(x, w_attn, b_attn, w_proj, b_proj)
    res = run_bass_kernel_spmd(nc, in_maps, core_ids=list(range(8)))
    _CACHED["last_results"] = res
    outs = [res.results[c]["out"] for c in range(8)]
    return _assemble_full(outs)
